# revision 34
# baseline (speedup 1.0000x reference)
"""GCNEncoder v5: dma_gather (4 SWDGE queues, 1024-idx instructions, the
ucode caps out between 1024 and 1536 idx/instruction) from a pair-packed
bf16 table viewed [NPAD/2, 128] (256B rows satisfy the elem%256B dma_gather
constraint; int16 idx = table_row>>1 with 2 windows of 32768) +
selection-matrix aggregation on PE.

Edges per core are sorted by (list, dst) where list = 2*(pair_row >= 32768)
+ (row & 1); chunks of 128 edges are (dst-block, list)-pure (padded with
w=0 dummies to a shared-across-cores chunk count S_bl so the SPMD program
is identical on all cores).  Per chunk one DVE tensor_scalar builds
S = (iota == dstloc) * w  [128 edges, 128 dst]  and one PE matmul
accumulates aggT[64, 128] += msg.T @ S in PSUM.  Per block: two W matmuls
(bf16), relu+bias, transpose, y write; one Shared-output AllGather between
layers (5.77ms baseline -> 3.9ms).

Measured on HW: dma_gather costs ~8ns/idx on one SWDGE queue (Q7
descriptor generation bound) but ~1.2-2.7ns/idx round-robined over 4
queues; multi-offset indirect_dma_start is impossible (the walrus/Q7
contract is one dynamic offset per partition, 128 descs/instruction);
the AllGather itself is ~40us but adds ~600us per layer boundary when
interleaved with the gather stream (pipeline drain + collective
rendezvous); splitting it into 4 per-block-range collectives makes it
worse (4 rendezvous per layer)."""

import os

import numpy as np

P = 128
D = 64
NCORES = 8
N = 100000
B = 98                     # dst blocks per core
CORE_N = B * P             # 12544
NPAD = NCORES * CORE_N     # 100352
NPAIR = NPAD // 2          # 50176
WIN = 32768
GQ = int(os.environ.get("GCN_GQ", "512"))  # idx per dma_gather instruction
NQUEUES = 4
SCRATCH = int(os.environ.get("GCN_DMA_SCRATCH", "32768"))
OOR = 999.0
SPLIT_BOUNDS = (0, 98)
NO_SBUILD = bool(int(os.environ.get("GCN_NO_SBUILD", "0")))
GATHER_ONLY = bool(int(os.environ.get("GCN_GATHER_ONLY", "0")))
NO_AG = bool(int(os.environ.get("GCN_NO_AG", "0")))
AG_ONLY = bool(int(os.environ.get("GCN_AG_ONLY", "0")))
XTAB_ALL = bool(int(os.environ.get("GCN_XTAB_ALL", "0")))                # dst-local marker that never matches iota 0..127


# ---------------------------------------------------------------- host prep


def _preprocess(x, edge_index, edge_weight):
    import ml_dtypes

    src = np.asarray(edge_index[0], dtype=np.int64)
    dst = np.asarray(edge_index[1], dtype=np.int64)
    w = np.asarray(edge_weight, dtype=np.float64)

    key = dst * N + src
    ukey, inv = np.unique(key, return_inverse=True)
    uw = np.bincount(inv, weights=w).astype(np.float32)
    udst = (ukey // N).astype(np.int64)
    usrc = (ukey % N).astype(np.int64)

    core = udst // CORE_N
    block = (udst % CORE_N) // P
    dstp = udst % P

    # split-aware table layout: 4 splits of blocks; each split's 8 rank
    # slices are contiguous so per-split AllGathers write contiguous rows
    BS = np.array(SPLIT_BOUNDS, dtype=np.int64)        # e.g. [0,25,50,74,98]
    rows_s = (BS[1:] - BS[:-1]) * P
    off_s = np.concatenate([[0], np.cumsum(rows_s * NCORES)[:-1]])

    def table_row(node):
        c = node // CORE_N
        loc = node % CORE_N
        blk = loc // P
        s = np.searchsorted(BS[1:], blk, side="right")
        return off_s[s] + c * rows_s[s] + (loc - BS[s] * P)

    new_row = table_row(usrc)
    pr = new_row >> 1
    par = new_row & 1
    wlist = (pr >= WIN).astype(np.int64) * 2 + par
    pr_local = pr - (pr >= WIN) * WIN

    # counts per (core, block, list) -> shared chunk structure S_bl
    cnt = np.zeros((NCORES, B, 4), dtype=np.int64)
    np.add.at(cnt, (core, block, wlist), 1)
    S_bl = np.maximum(1, -(-cnt.max(axis=0) // P))  # [B, 4] chunks
    cap = S_bl * P

    # chunk-column offsets per (b, l) within each list's concatenated stream
    chunk_off = np.zeros((B, 4), dtype=np.int64)  # chunk index within list l
    for l in range(4):
        chunk_off[:, l] = np.concatenate([[0], np.cumsum(S_bl[:, l])[:-1]])
    nchunks_l = S_bl.sum(axis=0)                   # chunks per list
    # pad each list's idx stream to a multiple of GQ
    idx_per_l = nchunks_l * P
    idx_per_l_pad = -(-idx_per_l // GQ) * GQ
    ngather_l = idx_per_l_pad // GQ
    col_off_l = np.concatenate([[0], np.cumsum(idx_per_l_pad // 16)])
    totcols = int(col_off_l[-1])

    nentries = int(nchunks_l.sum())

    # per-core data arrays
    idx_tab = np.zeros((NCORES, 16, totcols), dtype=np.int16)
    dst_tab = np.full((NCORES, P, nentries), OOR, dtype=np.float32)
    w_tab = np.zeros((NCORES, P, nentries), dtype=np.float32)

    # entry index for (b, l, s): entries laid out list-major then block
    ent_off = np.zeros((B, 4), dtype=np.int64)
    ent_base = np.concatenate([[0], np.cumsum(nchunks_l)[:-1]])
    for l in range(4):
        ent_off[:, l] = ent_base[l] + chunk_off[:, l]

    order = np.lexsort((udst, wlist, block, core))
    group_key = (core[order] * B + block[order]) * 4 + wlist[order]
    assert np.all(np.diff(group_key) >= 0)
    bounds = {}
    uniq, starts = np.unique(group_key, return_index=True)
    starts = np.append(starts, len(order))
    for gi, g in enumerate(uniq):
        bounds[int(g)] = (starts[gi], starts[gi + 1])

    for c in range(NCORES):
        for b in range(B):
            for l in range(4):
                g = (c * B + b) * 4 + l
                if g in bounds:
                    s, e = bounds[g]
                    eids = order[s:e]
                else:
                    eids = np.empty(0, dtype=np.int64)
                n = len(eids)
                assert n <= cap[b, l]
                base_chunk = chunk_off[b, l]
                j = np.arange(n)
                gpos = base_chunk * P + j          # position in list stream
                colg = col_off_l[l] + gpos // 16
                idx_tab[c, gpos % 16, colg] = pr_local[eids].astype(np.int16)
                entc = ent_off[b, l] + j // P
                dst_tab[c, j % P, entc] = dstp[eids].astype(np.float32)
                w_tab[c, j % P, entc] = uw[eids]

    x32 = np.asarray(x, dtype=np.float32)
    x_new = np.zeros((NPAD, D), dtype=np.float32)
    x_new[:N] = x32
    x_perm = np.zeros((NPAD, D), dtype=np.float32)
    allrows = table_row(np.arange(NPAD, dtype=np.int64))
    x_perm[allrows] = x_new
    x_tab = x_perm.astype(ml_dtypes.bfloat16).reshape(NPAIR, 2 * D)
    xT = np.ascontiguousarray(
        x_new.reshape(NCORES, CORE_N, D).transpose(0, 2, 1)
    ).astype(ml_dtypes.bfloat16)

    iota = np.tile(np.arange(P, dtype=np.float32), (P, 1)).astype(ml_dtypes.bfloat16)

    # per-chunk parity (list-major layout, same for all cores)
    par_of_list = [0, 1, 0, 1]

    return dict(
        S_bl=S_bl,
        chunk_off=chunk_off,
        nchunks_l=nchunks_l,
        ngather_l=ngather_l,
        col_off_l=col_off_l,
        totcols=totcols,
        nentries=nentries,
        ent_off=ent_off,
        idx_tab=idx_tab,
        dst_tab=dst_tab,
        w_tab=w_tab,
        x_tab=np.ascontiguousarray(x_tab),
        xT=xT,
        iota=iota,
        par_of_list=par_of_list,
    )


# ---------------------------------------------------------------- bass build


def _build(prep, W=None, repeat=1):
    import concourse.bacc as bacc
    import concourse.bass as bass
    import concourse.mybir as mybir
    import concourse.tile as tile
    from concourse.masks import make_identity

    f32 = mybir.dt.float32
    bf16 = mybir.dt.bfloat16
    i16 = mybir.dt.int16

    S_bl = prep["S_bl"]
    chunk_off = prep["chunk_off"]
    ngather_l = prep["ngather_l"]
    col_off_l = prep["col_off_l"]
    totcols = prep["totcols"]
    nentries = prep["nentries"]
    ent_off = prep["ent_off"]
    par_of_list = prep["par_of_list"]

    nc = bacc.Bacc(
        "TRN2",
        target_bir_lowering=False,
        debug=False,
        num_devices=NCORES,
        num_swdge_queues=NQUEUES,
        dynamic_dma_scratch_size=SCRATCH,
    )

    x_tab_in = nc.dram_tensor("x_tab", [NPAD, D], bf16, kind="ExternalInput")
    xT_in = nc.dram_tensor("xT", [D, CORE_N], bf16, kind="ExternalInput")
    idx_in = nc.dram_tensor("idx_tab", [16, totcols], i16, kind="ExternalInput")
    dst_in = nc.dram_tensor("dst_tab", [P, nentries], f32, kind="ExternalInput")
    w_in = nc.dram_tensor("w_tab", [P, nentries], f32, kind="ExternalInput")
    iota_in = nc.dram_tensor("iota", [P, P], bf16, kind="ExternalInput")
    wmat_in = {
        nm: nc.dram_tensor(nm, [D, D], bf16, kind="ExternalInput")
        for nm in ("W_rel1", "W_root1", "W_rel2", "W_root2", "W_rel3", "W_root3")
    }
    bias_in = {
        nm: nc.dram_tensor(nm, [D, 1], f32, kind="ExternalInput")
        for nm in ("b1", "b2", "b3")
    }
    out_t = nc.dram_tensor("h3", [CORE_N, D], f32, kind="ExternalOutput")

    with tile.TileContext(nc) as tc:
        with (
            tc.tile_pool(name="const", bufs=1) as cpool,
            tc.tile_pool(name="dram", bufs=1, space="DRAM") as dpool,
            tc.tile_pool(name="gather", bufs=10) as gpool,
            tc.tile_pool(name="sel", bufs=8) as spool,
            tc.tile_pool(name="work", bufs=4) as wpool,
            tc.tile_pool(name="psum", bufs=1, space="PSUM") as ppool,
        ):
            idx_res = cpool.tile([P, totcols], i16, tag="idx")
            dst_res = cpool.tile([P, nentries], f32, tag="dst")
            w_res = cpool.tile([P, nentries], f32, tag="w")
            iota_t = cpool.tile([P, P], bf16, tag="iota")
            ident = cpool.tile([P, P], f32, tag="ident")
            ident_bf = cpool.tile([P, P], bf16, tag="ident_bf")
            hT_a = cpool.tile([D, CORE_N], bf16, tag="hT_a")
            hT_b = cpool.tile([D, CORE_N], bf16, tag="hT_b")
            Wt = {k: cpool.tile([D, D], bf16, tag=k, name=k) for k in wmat_in}
            bt = {k: cpool.tile([D, 1], f32, tag=k, name=k) for k in bias_in}

            # idx input is [16, totcols]; replicate into all 8 groups of 16
            # partitions (the gather ucode reads its own core's copy).
            for r in range(8):
                nc.sync.dma_start(
                    out=idx_res[16 * r:16 * (r + 1), :], in_=idx_in.ap()
                )
            nc.sync.dma_start(out=dst_res[:], in_=dst_in.ap())
            nc.sync.dma_start(out=w_res[:], in_=w_in.ap())
            nc.sync.dma_start(out=iota_t[:], in_=iota_in.ap())
            for k in Wt:
                nc.sync.dma_start(out=Wt[k][:], in_=wmat_in[k].ap())
            for k in bt:
                nc.sync.dma_start(out=bt[k][:], in_=bias_in[k].ap())
            make_identity(nc, ident[:])
            nc.vector.tensor_copy(out=ident_bf[:], in_=ident[:])
            S_const = cpool.tile([P, P], bf16, tag="S_const")
            nc.vector.memset(S_const[:], 0.0)

            BS = SPLIT_BOUNDS
            y_split = [
                dpool.tile([(BS[s + 1] - BS[s]) * P, D], bf16,
                           tag=f"ys{s}", name=f"ys{s}")
                for s in range(len(BS) - 1)
            ]

            for _rep in range(repeat):
                tables = [
                    dpool.tile([NPAD, D], bf16, tag=f"tab{l}_{_rep}",
                               name=f"tab{l}_{_rep}", addr_space="Shared")
                    for l in (1, 2)
                ]
                nc.sync.dma_start(out=hT_a[:], in_=xT_in.ap())
                _layers(nc, bass, mybir, prep, locals())

    nc.compile()
    return nc


def _layers(nc, bass, mybir, prep, env):
    f32 = mybir.dt.float32
    bf16 = mybir.dt.bfloat16

    S_bl = prep["S_bl"]
    chunk_off = prep["chunk_off"]
    ngather_l = prep["ngather_l"]
    col_off_l = prep["col_off_l"]
    ent_off = prep["ent_off"]
    par_of_list = prep["par_of_list"]

    gpool = env["gpool"]
    spool = env["spool"]
    wpool = env["wpool"]
    ppool = env["ppool"]
    idx_res = env["idx_res"]
    dst_res = env["dst_res"]
    w_res = env["w_res"]
    iota_t = env["iota_t"]
    ident = env["ident"]
    ident_bf = env["ident_bf"]
    hT_a = env["hT_a"]
    hT_b = env["hT_b"]
    Wt = env["Wt"]
    bt = env["bt"]
    tables = env["tables"]
    y_split = env["y_split"]
    BS = SPLIT_BOUNDS
    x_tab_in = env["x_tab_in"]
    out_t = env["out_t"]

    qcounter = [0]

    for layer in (1, 2, 3):
        src_ap = x_tab_in.ap() if (layer == 1 or XTAB_ALL) else tables[layer - 2][:]
        pairv = src_ap.rearrange("(n two) d -> n (two d)", two=2)
        tab_w0 = pairv[0:WIN, :]
        tab_w1 = pairv[WIN:NPAIR, :]
        tabs = [tab_w0, tab_w0, tab_w1, tab_w1]
        hT_prev = hT_a if layer % 2 == 1 else hT_b
        hT_cur = hT_b if layer % 2 == 1 else hT_a
        W_rel = Wt[f"W_rel{layer}"]
        W_root = Wt[f"W_root{layer}"]
        b_l = bt[f"b{layer}"]

        # rolling gather state per list
        g_tiles = [[], [], [], []]   # (first_chunk, tile)
        issued = [0, 0, 0, 0]        # gathers issued per list

        def issue_gather(l):
            gidx = issued[l]
            assert gidx < ngather_l[l]
            g = gpool.tile([P, (GQ // P) * 2 * D], bf16, tag=f"g{l}")
            nc.gpsimd.dma_gather(
                out_ap=g[:].rearrange("p (c e) -> p c e", e=2 * D),
                in_ap=tabs[l],
                idxs_ap=idx_res[:, col_off_l[l] + gidx * (GQ // 16):
                                col_off_l[l] + (gidx + 1) * (GQ // 16)],
                num_idxs=GQ,
                num_idxs_reg=GQ,
                elem_size=2 * D,
                queue_num=qcounter[0] % NQUEUES,
            )
            qcounter[0] += 1
            g_tiles[l].append((gidx * (GQ // P), g))
            if len(g_tiles[l]) > 10:
                g_tiles[l].pop(0)
            issued[l] += 1

        def need_chunk(l, ch):
            """Return (tile, slot) covering chunk ch of list l, issuing
            gathers as needed."""
            while issued[l] * (GQ // P) <= ch:
                issue_gather(l)
            for first, g in g_tiles[l]:
                if first <= ch < first + GQ // P:
                    return g, ch - first
            raise AssertionError("gather tile evicted too early")

        # prefetch first gather of each list
        if not AG_ONLY:
            for l in range(4):
                issue_gather(l)

        for b in range(B):
            if AG_ONLY:
                break
            # prefetch: ensure gathers covering through block b+3 are issued
            bn = min(b + 6, B - 1)
            for l in range(4):
                tgt = int(chunk_off[bn, l]) + int(S_bl[bn, l])
                while issued[l] * (GQ // P) < tgt and issued[l] < int(ngather_l[l]):
                    issue_gather(l)
            ps_agg = ppool.tile([D, P], f32, tag="agg", bufs=2)
            first = True
            total_entries = int(S_bl[b].sum())
            done = 0
            for l in range(4):
                par = par_of_list[l]
                for s in range(int(S_bl[b, l])):
                    ch = int(chunk_off[b, l]) + s
                    g, slot = need_chunk(l, ch)
                    if GATHER_ONLY:
                        continue
                    ecol = int(ent_off[b, l]) + s
                    if NO_SBUILD:
                        S_t = env["S_const"]
                    else:
                        S_t = spool.tile([P, P], bf16, tag="S")
                        nc.vector.tensor_scalar(
                            out=S_t[:],
                            in0=iota_t[:],
                            scalar1=dst_res[:, ecol:ecol + 1],
                            scalar2=w_res[:, ecol:ecol + 1],
                            op0=mybir.AluOpType.is_equal,
                            op1=mybir.AluOpType.mult,
                        )
                    done += 1
                    nc.tensor.matmul(
                        out=ps_agg[:],
                        lhsT=g[:, slot * 2 * D + par * D: slot * 2 * D + (par + 1) * D],
                        rhs=S_t[:],
                        start=first,
                        stop=(done == total_entries),
                    )
                    first = False

            if GATHER_ONLY:
                continue
            sT = wpool.tile([D, P], bf16, tag="sT")
            nc.scalar.activation(
                out=sT[:], in_=ps_agg[:],
                func=mybir.ActivationFunctionType.Copy,
            )
            ps2 = ppool.tile([D, P], f32, tag="ps2", bufs=2)
            nc.tensor.matmul(
                out=ps2[:], lhsT=W_rel[:], rhs=sT[:], start=True, stop=False
            )
            nc.tensor.matmul(
                out=ps2[:], lhsT=W_root[:],
                rhs=hT_prev[:, b * P:(b + 1) * P],
                start=False, stop=True,
            )
            if layer < 3:
                nc.scalar.activation(
                    out=hT_cur[:, b * P:(b + 1) * P],
                    in_=ps2[:],
                    func=mybir.ActivationFunctionType.Relu,
                    bias=b_l[:],
                )
                hb = ppool.tile([P, D], bf16, tag="hb", bufs=2)
                nc.tensor.transpose(
                    out=hb[:],
                    in_=hT_cur[:, b * P:(b + 1) * P],
                    identity=ident_bf[:D, :D],
                )
                hs = wpool.tile([P, D], bf16, tag="hs")
                nc.scalar.activation(
                    out=hs[:], in_=hb[:],
                    func=mybir.ActivationFunctionType.Copy,
                )
                s = next(i for i in range(len(BS) - 1) if BS[i] <= b < BS[i + 1])
                nc.sync.dma_start(
                    out=y_split[s][(b - BS[s]) * P:(b - BS[s] + 1) * P, :],
                    in_=hs[:],
                )
                if layer < 3 and b == BS[s + 1] - 1 and not NO_AG:
                    rows = (BS[s + 1] - BS[s]) * P
                    goff = 0
                    for i in range(s):
                        goff += (BS[i + 1] - BS[i]) * P * NCORES
                    nc.gpsimd.collective_compute(
                        "AllGather",
                        mybir.AluOpType.bypass,
                        replica_groups=[list(range(NCORES))],
                        ins=[y_split[s][:].opt()],
                        outs=[tables[layer - 1][:][goff:goff + rows * NCORES, :].opt()],
                    )
            else:
                oT = wpool.tile([D, P], f32, tag="oT")
                nc.scalar.activation(
                    out=oT[:], in_=ps2[:],
                    func=mybir.ActivationFunctionType.Identity,
                    bias=b_l[:],
                )
                ob = ppool.tile([P, D], f32, tag="ob", bufs=2)
                nc.tensor.transpose(out=ob[:], in_=oT[:], identity=ident[:D, :D])
                os_ = wpool.tile([P, D], f32, tag="os")
                nc.scalar.activation(
                    out=os_[:], in_=ob[:],
                    func=mybir.ActivationFunctionType.Copy,
                )
                nc.sync.dma_start(
                    out=out_t.ap()[b * P:(b + 1) * P, :], in_=os_[:]
                )



# ---------------------------------------------------------------- entry


def _prep_and_build(inputs):
    import ml_dtypes

    prep = _preprocess(inputs["x"], inputs["edge_index"], inputs["edge_weight"])
    W = {
        k: np.ascontiguousarray(np.asarray(inputs[k], dtype=np.float32))
        for k in (
            "W_rel1", "b_rel1", "W_root1",
            "W_rel2", "b_rel2", "W_root2",
            "W_rel3", "b_rel3", "W_root3",
        )
    }
    nc = _build(prep, W)
    in_maps = []
    for c in range(NCORES):
        m = {
            "x_tab": prep["x_tab"],
            "xT": prep["xT"][c],
            "idx_tab": np.ascontiguousarray(prep["idx_tab"][c]),
            "dst_tab": np.ascontiguousarray(prep["dst_tab"][c]),
            "w_tab": np.ascontiguousarray(prep["w_tab"][c]),
            "iota": prep["iota"],
        }
        for l in (1, 2, 3):
            m[f"W_rel{l}"] = W[f"W_rel{l}"].astype(ml_dtypes.bfloat16)
            m[f"W_root{l}"] = W[f"W_root{l}"].astype(ml_dtypes.bfloat16)
            m[f"b{l}"] = W[f"b_rel{l}"].reshape(D, 1)
        in_maps.append(m)
    return prep, nc, in_maps


def _reassemble(prep, core_outs):
    out = np.concatenate([core_outs[c] for c in range(NCORES)], axis=0)
    return np.ascontiguousarray(out[:N])


def kernel(**inputs) -> np.ndarray:
    from concourse.bass_utils import run_bass_kernel_spmd

    prep, nc, in_maps = _prep_and_build(inputs)
    res = run_bass_kernel_spmd(
        nc,
        in_maps,
        core_ids=list(range(NCORES)),
        trace=bool(int(os.environ.get("GCN_TRACE", "0"))),
    )
    kernel.last_results = res
    kernel.last_nc = nc
    kernel.last_in_maps = in_maps
    kernel.last_prep = prep
    return _reassemble(prep, [res.results[c]["h3"] for c in range(NCORES)])


if __name__ == "__main__":
    import reference

    inputs = {k: np.asarray(v) for k, v in reference.setup_inputs().items()}
    expected = np.asarray(reference.reference(**inputs))
    actual = kernel(**inputs)
    err = np.abs(actual - expected).max() / (np.abs(expected).max() + 1e-9)
    rel = np.linalg.norm(actual - expected) / (np.linalg.norm(expected) + 1e-30)
    print("max-abs-rel:", err, " fro-rel:", rel)
